# revision 48
# baseline (speedup 1.0000x reference)
"""Trainium2 Bass kernel for nn_Joint (dense transformer block), 8 NeuronCores.

Sharding: 8 cores = 4 batches x 2 sequence halves. Each core computes the
full MLP->h and K/V projections for its batch (duplicated inside the pair,
no collectives), but only its own 1024-token half of queries / attention
rows / FFN / output. Token "roll" trick: each core's x is rotated so its own
half is always tokens [0:1024]; attention over all 2048 keys is
permutation-invariant, so the same SPMD program works for both halves.

v2: transposed-scores attention. Scores are computed directly as
S^T = kT.T @ qT (keys on partitions), so P^T = exp(S^T * scale) lands in
SBUF already in the layout attn_out needs as lhsT -- no PE transposes or
PSUM->SBUF copies in the softmax path. Row sums come from an extra N=1
matmul per key-block reusing the loaded stationary operand; softmax skips
max-subtraction (scores*scale bounded by ~0.35 for this model, exp cannot
overflow). LayerNorm math uses fused scalar_tensor_tensor /
tensor_tensor_reduce ops in bf16; the scalar engine runs Exp-only during
attention (Sqrt batched once per 512-query block) to avoid act-table
thrashing. Weight DMAs are prefetched a phase early on the sync queue;
xT streams on gpsimd. SBUF pools use explicit left/right stack lifetimes.

Layouts on chip (per core):
  xT   [768, 2048]  bf16  feature-major (host pre-transposed)
  hT   [568(+1), 2048] bf16 feature-major; row 56 of chunk 4 = ones (bias row)
  kT   [1024, 2048] bf16  feature-major
  qT   [1024, 1024] bf16  feature-major (own half)
  V    [2048, 1024] bf16  token-major
  xmod [1024, 1024] bf16  token-major, bm+bv folded in via the ones row
  PT   [2048, 512]  bf16  exp(S^T*scale) per 512-query block, keys on partitions
  x1   [1024, 1024] bf16  token-major; x1T via PE transpose for FFN
All matmuls bf16 inputs with fp32 PSUM accumulation; softmax/LN math fp32.
"""

import sys

if "/opt/trn_rl_repo" not in sys.path:
    sys.path.insert(0, "/opt/trn_rl_repo")

import numpy as np
import ml_dtypes

import concourse.bass as bass
import concourse.mybir as mybir
import concourse.tile as tile
from concourse import bacc
from concourse.masks import make_identity

BF16 = mybir.dt.bfloat16
F32 = mybir.dt.float32
AF = mybir.ActivationFunctionType
ALU = mybir.AluOpType
AX = mybir.AxisListType

B, S, IN_C, HID, D = 4, 2048, 768, 568, 1024
Q = S // 2  # own-half query tokens per core
KB = S // 128  # 16 key blocks
EPS = 1e-5
SCALE = 1.0 / np.sqrt(np.float32(D))  # 1/32
NCORES = 8

# K-chunking of the HID=568 contraction: 4x128 + 56 (+1 ones row for wm/wv)
HID_CH = [128, 128, 128, 128, 56]
HID_CH_AUG = [128, 128, 128, 128, 57]


def build_program():
    nc = bacc.Bacc("TRN2")

    # ---- DRAM I/O ----
    xT = nc.dram_tensor("xT", [IN_C, S], BF16, kind="ExternalInput")
    w_mlp = nc.dram_tensor("w_mlp", [IN_C, HID], BF16, kind="ExternalInput")
    wq = nc.dram_tensor("wq", [HID, D], BF16, kind="ExternalInput")
    wk = nc.dram_tensor("wk", [HID, D], BF16, kind="ExternalInput")
    # wv/wm are host-augmented to HID+1 rows: wv gets a zero row, wm gets
    # bm+bv -- the hT ones-row turns that into a free bias add
    wv = nc.dram_tensor("wv", [HID + 1, D], BF16, kind="ExternalInput")
    wm = nc.dram_tensor("wm", [HID + 1, D], BF16, kind="ExternalInput")
    wf1 = nc.dram_tensor("wf1", [D, D], BF16, kind="ExternalInput")
    wf2 = nc.dram_tensor("wf2", [D, D], BF16, kind="ExternalInput")
    b_mlp = nc.dram_tensor("b_mlp", [HID], F32, kind="ExternalInput")
    bq = nc.dram_tensor("bq", [D], F32, kind="ExternalInput")
    bk = nc.dram_tensor("bk", [D], F32, kind="ExternalInput")
    bf1 = nc.dram_tensor("bf1", [D], F32, kind="ExternalInput")
    bf2 = nc.dram_tensor("bf2", [D], BF16, kind="ExternalInput")
    g1 = nc.dram_tensor("g1", [D], BF16, kind="ExternalInput")
    be1 = nc.dram_tensor("be1", [D], BF16, kind="ExternalInput")
    g2 = nc.dram_tensor("g2", [D], BF16, kind="ExternalInput")
    be2 = nc.dram_tensor("be2", [D], BF16, kind="ExternalInput")
    y = nc.dram_tensor("y", [Q, D], F32, kind="ExternalOutput")

    def bcast_ap(handle, n):
        a = handle[:]
        return bass.AP(tensor=a.tensor, offset=a.offset, ap=[[0, 128]] + list(a.ap))

    with tile.TileContext(nc) as tc:
        with (
            tc.tile_pool(name="singles", bufs=1) as singles,
            tc.tile_pool(name="x1_pool", bufs=1) as x1_pool,
        ):
            # ---------- constants / biases (small, left stack bottom) ----------
            ident = singles.tile([128, 128], BF16)
            make_identity(nc, ident)
            eps_t = singles.tile([128, 1], F32)
            nc.vector.memset(eps_t, EPS)
            ones_sb = singles.tile([128, 2], BF16)
            nc.vector.memset(ones_sb, 1.0)

            bmlp_sb = singles.tile([128, 5], F32)
            for m in range(5):
                m0 = m * 128
                msz = HID_CH[m]
                nc.sync.dma_start(
                    out=bmlp_sb[:msz, m : m + 1],
                    in_=b_mlp[m0 : m0 + msz].rearrange("(a b) -> a b", b=1),
                )
            bq_sb = singles.tile([128, 8], F32)
            nc.sync.dma_start(out=bq_sb, in_=bq.rearrange("(c p) -> p c", p=128))
            bk_sb = singles.tile([128, 8], F32)
            nc.sync.dma_start(out=bk_sb, in_=bk.rearrange("(c p) -> p c", p=128))
            bf1_sb = singles.tile([128, 8], F32)
            nc.sync.dma_start(out=bf1_sb, in_=bf1.rearrange("(c p) -> p c", p=128))

            # small LN scratch
            mu4 = singles.tile([128, 4], F32)
            sq4 = singles.tile([128, 4], F32)
            ve4 = singles.tile([128, 4], F32)
            msqe4 = singles.tile([128, 4], F32)
            std4 = singles.tile([128, 4], F32)
            rstd4 = singles.tile([128, 4], F32)
            rercp4 = singles.tile([128, 4], F32)
            sa0 = singles.tile([128, 1], F32)
            sa1 = singles.tile([128, 1], F32)
            musum = singles.tile([128, 1], F32)
            bf2sum = singles.tile([128, 1], F32)
            sqa = singles.tile([128, 1], F32)
            mu2 = singles.tile([128, 1], F32)
            msqe2 = singles.tile([128, 1], F32)
            ve2 = singles.tile([128, 1], F32)
            std2 = singles.tile([128, 1], F32)
            rstd2 = singles.tile([128, 1], F32)

            x1_sb = [x1_pool.tile([128, D], BF16, tag=f"x1_{i}", name=f"x1_{i}") for i in range(8)]

            # ---------- pools: left stack (phase-scoped), right stack (late) ----------
            x1T_pool = tc.alloc_tile_pool(name="x1T", bufs=1, side="right")
            x1T = x1T_pool.tile([128, 8, Q], BF16, name="x1T")

            kqvm = tc.alloc_tile_pool(name="kqvm", bufs=1, side="left")
            kT_sb = [kqvm.tile([128, S], BF16, tag=f"kT_{i}", name=f"kT_{i}") for i in range(8)]
            qT_sb = [kqvm.tile([128, Q], BF16, tag=f"qT_{i}", name=f"qT_{i}") for i in range(8)]
            v_sb = [kqvm.tile([128, D], BF16, tag=f"v_{i}", name=f"v_{i}") for i in range(16)]
            xm_sb = [kqvm.tile([128, D], BF16, tag=f"xm_{i}", name=f"xm_{i}") for i in range(8)]

            hT_pool = tc.alloc_tile_pool(name="hT", bufs=1, side="left")
            hT_sb = [hT_pool.tile([128, S], BF16, tag=f"hT_{i}", name=f"hTs_{i}") for i in range(5)]

            stream = tc.alloc_tile_pool(name="stream", bufs=1, side="left")
            xw_pool = tc.alloc_tile_pool(name="xw", bufs=1, side="left")
            wmlp_sb = xw_pool.tile([128, 6, HID], BF16, name="wmlp_sb")

            pp_sc = tc.alloc_tile_pool(name="psum_sc", bufs=1, space="PSUM")
            pp_mm = tc.alloc_tile_pool(name="psum_mm", bufs=1, space="PSUM")

            # ---------- phase 0: hT = relu(w_mlp.T @ xT + b_mlp) ----------
            nc.sync.dma_start(
                out=wmlp_sb, in_=w_mlp.rearrange("(c p) h -> p c h", p=128)
            )
            # ones row for the bias-via-matmul trick (xm / V): memset a
            # 32-aligned partition band; the mlp activations later overwrite
            # rows 32..55, leaving row 56 = 1.0
            nc.gpsimd.memset(hT_sb[4][32:64, :], 1.0)

            # streamed projection weights; two tag sets alternate so the next
            # projection's DMA overlaps the current one's matmuls
            def load_w(wdram, st, chunks=HID_CH):
                tiles = []
                for i in range(5):
                    i0, isz = i * 128, chunks[i]
                    t = stream.tile([128, D], BF16, tag=f"wp{st}_{i}", name=f"wp{st}_{i}")
                    nc.sync.dma_start(out=t[:isz], in_=wdram[i0 : i0 + isz, :])
                    tiles.append(t)
                return tiles

            wk_sb = load_w(wk, 0)
            wq_sb = load_w(wq, 1)

            # xT streamed in 512-column waves, one 3D DMA per wave
            for n in range(4):
                ns = bass.ts(n, 512)
                xs = stream.tile([128, 6, 512], BF16, tag="xs", name="xs", bufs=2)
                for i in range(6):
                    eng = nc.gpsimd if i % 2 == 0 else nc.scalar
                    eng.dma_start(
                        out=xs[:, i, :],
                        in_=xT[i * 128 : (i + 1) * 128, n * 512 : (n + 1) * 512],
                    )
                for m in range(5):
                    m0, msz = m * 128, HID_CH[m]
                    ps = pp_mm.tile([128, 512], F32, tag="mm", bufs=4)
                    for kk in range(6):
                        nc.tensor.matmul(
                            ps[:msz],
                            wmlp_sb[:, kk, m0 : m0 + msz],
                            xs[:, kk, :],
                            start=(kk == 0),
                            stop=(kk == 5),
                        )
                    nc.scalar.activation(
                        out=hT_sb[m][:msz, ns],
                        in_=ps[:msz],
                        func=AF.Relu,
                        bias=bmlp_sb[:msz, m : m + 1],
                    )
            xw_pool.release()

            # ---------- phase 1: projections ----------
            for m in range(8):
                ms = bass.ts(m, 128)
                for n in range(4):
                    ns = bass.ts(n, 512)
                    ps = pp_mm.tile([128, 512], F32, tag="mm", bufs=4)
                    for kk in range(5):
                        ksz = HID_CH[kk]
                        nc.tensor.matmul(
                            ps,
                            wk_sb[kk][:ksz, ms],
                            hT_sb[kk][:ksz, ns],
                            start=(kk == 0),
                            stop=(kk == 4),
                        )
                    nc.scalar.activation(
                        out=kT_sb[m][:, ns], in_=ps, func=AF.Identity,
                        bias=bk_sb[:, m : m + 1],
                    )
            wv_sb = load_w(wv, 0, HID_CH_AUG)  # reuses wk's buffers once kT drains
            for m in range(8):
                ms = bass.ts(m, 128)
                for n in range(2):
                    ns = bass.ts(n, 512)
                    ps = pp_mm.tile([128, 512], F32, tag="mm", bufs=4)
                    for kk in range(5):
                        ksz = HID_CH[kk]
                        nc.tensor.matmul(
                            ps,
                            wq_sb[kk][:ksz, ms],
                            hT_sb[kk][:ksz, ns],
                            start=(kk == 0),
                            stop=(kk == 4),
                        )
                    nc.scalar.activation(
                        out=qT_sb[m][:, ns], in_=ps, func=AF.Identity,
                        bias=bq_sb[:, m : m + 1],
                    )

            def scores_block(b, pt):
                """S^T = kT.T @ qT for 512 queries; P^T = exp(S^T*scale) in SBUF."""
                qs = bass.ts(b, 512)
                for kb in range(KB):
                    ps = pp_sc.tile([128, 512], F32, tag="sc", bufs=2)
                    for kk in range(8):
                        nc.tensor.matmul(
                            ps,
                            kT_sb[kk][:, kb * 128 : (kb + 1) * 128],
                            qT_sb[kk][:, qs],
                            start=(kk == 0),
                            stop=(kk == 7),
                        )
                    nc.scalar.activation(
                        out=pt[:, kb, :], in_=ps, func=AF.Exp, scale=float(SCALE),
                    )

            pt_pool = tc.alloc_tile_pool(name="pt", bufs=1, side="right")
            pt = pt_pool.tile([128, KB, 512], BF16, name="pt")
            scores_block(0, pt)

            # V (token-major): wv row 56 of chunk 4 is zero (host-augmented);
            # wm row 56 carries bm+bv, added via the hT ones row
            wmm_sb = load_w(wm, 1, HID_CH_AUG)
            for m in range(16):
                ms = bass.ts(m, 128)
                for n in range(2):
                    ns = bass.ts(n, 512)
                    ps = pp_mm.tile([128, 512], F32, tag="mm", bufs=4)
                    for kk in range(5):
                        ksz = HID_CH_AUG[kk]
                        nc.tensor.matmul(
                            ps,
                            hT_sb[kk][:ksz, ms],
                            wv_sb[kk][:ksz, ns],
                            start=(kk == 0),
                            stop=(kk == 4),
                        )
                    nc.vector.tensor_copy(v_sb[m][:, ns], ps)
            # xmod (token-major, own half) + (bm+bv) via ones row
            for m in range(8):
                ms = bass.ts(m, 128)
                for n in range(2):
                    ns = bass.ts(n, 512)
                    ps = pp_mm.tile([128, 512], F32, tag="mm", bufs=4)
                    for kk in range(5):
                        ksz = HID_CH_AUG[kk]
                        nc.tensor.matmul(
                            ps,
                            hT_sb[kk][:ksz, ms],
                            wmm_sb[kk][:ksz, ns],
                            start=(kk == 0),
                            stop=(kk == 4),
                        )
                    nc.vector.tensor_copy(xm_sb[m][:, ns], ps)

            stream.release()
            hT_pool.release()
            pp_mm.release()

            # ---------- phase 2: attention ----------
            scratch = tc.alloc_tile_pool(name="scratch", bufs=1, side="right")
            tmpA = scratch.tile([128, D], BF16, name="tmpA")
            tmpB = scratch.tile([128, D], BF16, name="tmpB")
            bf2_b = scratch.tile([128, D], BF16, name="bf2_b")
            nc.gpsimd.dma_start(out=bf2_b, in_=bcast_ap(bf2, D))
            g1_b = scratch.tile([128, D], BF16, name="g1_b")
            nc.gpsimd.dma_start(out=g1_b, in_=bcast_ap(g1, D))
            be1_b = scratch.tile([128, D], BF16, name="be1_b")
            nc.gpsimd.dma_start(out=be1_b, in_=bcast_ap(be1, D))
            g2_b = scratch.tile([128, D], BF16, name="g2_b")
            nc.gpsimd.dma_start(out=g2_b, in_=bcast_ap(g2, D))
            be2_b = scratch.tile([128, D], BF16, name="be2_b")
            nc.gpsimd.dma_start(out=be2_b, in_=bcast_ap(be2, D))

            # precompute sum(bf2) for the LN2 mean correction
            nc.vector.tensor_reduce(
                out=bf2sum, in_=bf2_b, op=ALU.add, axis=AX.X
            )

            wf_pool = tc.alloc_tile_pool(name="wf", bufs=1, side="right")
            wf1_sb = wf_pool.tile([128, 8, D], BF16, name="wf1_sb")
            wf2_sb = wf_pool.tile([128, 8, D], BF16, name="wf2_sb")
            nc.sync.dma_start(
                out=wf1_sb, in_=wf1.rearrange("(c p) d -> p c d", p=128)
            )

            pp_at = tc.alloc_tile_pool(name="psum_at", bufs=1, space="PSUM")
            pp_rs = tc.alloc_tile_pool(name="psum_rs", bufs=1, space="PSUM")

            def attn_block(b, pt):
                for qc in range(4):
                    qi = b * 4 + qc
                    ms = qc * 128
                    ps0 = pp_at.tile([128, 512], F32, tag="at", bufs=3)
                    ps1 = pp_at.tile([128, 512], F32, tag="at", bufs=3)
                    # per-qc rowsum tile: a whole PSUM bank, so the DVE read
                    # below never shares a bank with the next qc's PE writes.
                    # N=2 keeps the matmul output 8-byte-cacheline aligned.
                    rs = pp_rs.tile([128, 2], F32, tag="rs", bufs=2)
                    for kb in range(KB):
                        lhsT = pt[:, kb, ms : ms + 128]
                        nc.tensor.matmul(
                            ps0, lhsT, v_sb[kb][:, 0:512],
                            start=(kb == 0), stop=(kb == KB - 1),
                        )
                        nc.tensor.matmul(
                            ps1, lhsT, v_sb[kb][:, 512:1024],
                            start=(kb == 0), stop=(kb == KB - 1),
                        )
                        nc.tensor.matmul(
                            rs, lhsT, ones_sb,
                            start=(kb == 0), stop=(kb == KB - 1),
                        )
                    nc.vector.reciprocal(rercp4[:, qc : qc + 1], rs[:, 0:1])
                    # x1pre = attn/rowsum + xmod; accum gives the LN mean sum
                    nc.vector.scalar_tensor_tensor(
                        out=x1_sb[qi][:, 0:512], in0=ps0,
                        scalar=rercp4[:, qc : qc + 1], in1=xm_sb[qi][:, 0:512],
                        op0=ALU.mult, op1=ALU.add, accum_out=sa0,
                    )
                    nc.vector.scalar_tensor_tensor(
                        out=x1_sb[qi][:, 512:1024], in0=ps1,
                        scalar=rercp4[:, qc : qc + 1], in1=xm_sb[qi][:, 512:1024],
                        op0=ALU.mult, op1=ALU.add, accum_out=sa1,
                    )
                    nc.vector.tensor_add(musum, sa0, sa1)
                    nc.vector.tensor_scalar_mul(mu4[:, qc : qc + 1], musum, 1.0 / D)
                    # sum of squares on the scalar engine (common act table,
                    # shared with the per-block Sqrt -- still 2 swaps/block)
                    nc.scalar.activation(
                        out=tmpB, in_=x1_sb[qi], func=AF.Square,
                        accum_out=sq4[:, qc : qc + 1],
                    )
                    nc.vector.scalar_tensor_tensor(
                        out=msqe4[:, qc : qc + 1], in0=mu4[:, qc : qc + 1],
                        scalar=mu4[:, qc : qc + 1], in1=eps_t,
                        op0=ALU.mult, op1=ALU.subtract,
                    )
                    nc.vector.scalar_tensor_tensor(
                        out=ve4[:, qc : qc + 1], in0=sq4[:, qc : qc + 1],
                        scalar=1.0 / D, in1=msqe4[:, qc : qc + 1],
                        op0=ALU.mult, op1=ALU.subtract,
                    )
                # batched rstd for the 4 chunks (one act-table swap per block)
                nc.scalar.activation(out=std4, in_=ve4, func=AF.Sqrt)
                nc.vector.reciprocal(rstd4, std4)
                for qc in range(4):
                    qi = b * 4 + qc
                    nc.vector.tensor_scalar(
                        out=tmpA, in0=x1_sb[qi],
                        scalar1=mu4[:, qc : qc + 1], scalar2=rstd4[:, qc : qc + 1],
                        op0=ALU.subtract, op1=ALU.mult,
                    )
                    nc.vector.tensor_mul(tmpB, tmpA, g1_b)
                    nc.vector.tensor_add(x1_sb[qi], tmpB, be1_b)

            def x1T_block(b, psum_pool, tp_bufs):
                for qc in range(4):
                    qi = b * 4 + qc
                    qoff = qi * 128
                    for g in range(2):
                        tp = psum_pool.tile([128, 512], BF16, tag="tp", bufs=tp_bufs)
                        for j in range(4):
                            dj = g * 4 + j
                            nc.tensor.transpose(
                                tp[:, j * 128 : (j + 1) * 128],
                                x1_sb[qi][:, dj * 128 : (dj + 1) * 128],
                                ident,
                            )
                        nc.vector.tensor_copy(
                            x1T[:, g * 4 : (g + 1) * 4, qoff : qoff + 128],
                            tp.rearrange("p (g q) -> p g q", q=128),
                        )

            attn_block(0, pt)
            scores_block(1, pt)
            x1T_block(0, pp_at, 1)
            attn_block(1, pt)
            pp_rs.release()
            pp_at.release()
            pp_sc.release()
            kqvm.release()

            # ---------- phase 3: FFN + LN2 + relu ----------
            pp_f = tc.alloc_tile_pool(name="psum_f", bufs=1, space="PSUM")
            f1T_pool = tc.alloc_tile_pool(name="f1T", bufs=2, side="left")
            ffn_t = tc.alloc_tile_pool(name="ffn_t", bufs=2, side="left")
            nc.sync.dma_start(
                out=wf2_sb, in_=wf2.rearrange("(c p) d -> p c d", p=128)
            )

            first = True
            for nch in range(2):
                f1T_sb = f1T_pool.tile([128, 8, 512], BF16, tag="f1T")
                for m in range(8):
                    ms = bass.ts(m, 128)
                    ps = pp_f.tile([128, 512], F32, tag="f", bufs=4)
                    for kk in range(8):
                        nc.tensor.matmul(
                            ps,
                            wf1_sb[:, kk, ms],
                            x1T[:, kk, nch * 512 : (nch + 1) * 512],
                            start=(kk == 0),
                            stop=(kk == 7),
                        )
                    nc.scalar.activation(
                        out=f1T_sb[:, m, :], in_=ps, func=AF.Relu,
                        bias=bf1_sb[:, m : m + 1],
                    )
                if first:
                    # block-1 x1T transposes, overlapped with f1 of nch 0
                    x1T_block(1, pp_f, 2)
                    first = False
                for tq in range(4):
                    qi = nch * 4 + tq
                    x2a = ffn_t.tile([128, D], BF16, tag="x2a")
                    x2pre = ffn_t.tile([128, D], BF16, tag="x2pre")
                    for dc in range(2):
                        ds_ = bass.ts(dc, 512)
                        ps = pp_f.tile([128, 512], F32, tag="f", bufs=4)
                        for kk in range(8):
                            nc.tensor.matmul(
                                ps,
                                f1T_sb[:, kk, tq * 128 : (tq + 1) * 128],
                                wf2_sb[:, kk, ds_],
                                start=(kk == 0),
                                stop=(kk == 7),
                            )
                        nc.vector.scalar_tensor_tensor(
                            out=x2a[:, ds_], in0=ps, scalar=0.0,
                            in1=x1_sb[qi][:, ds_], op0=ALU.bypass, op1=ALU.add,
                            accum_out=(sa0 if dc == 0 else sa1),
                        )
                    nc.vector.tensor_add(x2pre, x2a, bf2_b)
                    # mean sum = sum(x2a) + sum(bf2)
                    nc.vector.tensor_add(musum, sa0, sa1)
                    nc.vector.tensor_add(musum, musum, bf2sum)
                    nc.vector.tensor_scalar_mul(mu2, musum, 1.0 / D)
                    nc.scalar.activation(
                        out=x2a, in_=x2pre, func=AF.Square, accum_out=sqa,
                    )
                    nc.vector.scalar_tensor_tensor(
                        out=msqe2, in0=mu2, scalar=mu2, in1=eps_t,
                        op0=ALU.mult, op1=ALU.subtract,
                    )
                    nc.vector.scalar_tensor_tensor(
                        out=ve2, in0=sqa, scalar=1.0 / D, in1=msqe2,
                        op0=ALU.mult, op1=ALU.subtract,
                    )
                    nc.scalar.activation(out=std2, in_=ve2, func=AF.Sqrt)
                    nc.vector.reciprocal(rstd2, std2)
                    t1 = ffn_t.tile([128, D], BF16, tag="t1")
                    t2 = ffn_t.tile([128, D], BF16, tag="t2")
                    # normalize on the scalar engine: (x - mu)*rstd as
                    # Identity(scale=rstd, bias=-mu*rstd)
                    bias2 = ffn_t.tile([128, 1], F32, tag="bias2")
                    nc.vector.tensor_scalar(
                        out=bias2, in0=mu2, scalar1=rstd2, scalar2=-1.0,
                        op0=ALU.mult, op1=ALU.mult,
                    )
                    nc.scalar.activation(
                        out=t1, in_=x2pre, func=AF.Identity,
                        bias=bias2, scale=rstd2,
                    )
                    nc.vector.tensor_mul(t2, t1, g2_b)
                    nc.vector.tensor_add(t1, t2, be2_b)
                    out_t = ffn_t.tile([128, D], F32, tag="out")
                    nc.scalar.activation(out=out_t, in_=t1, func=AF.Relu)
                    if qi % 2 == 0:
                        nc.sync.dma_start(out=y[bass.ts(qi, 128), :], in_=out_t)
                    else:
                        nc.scalar.dma_start(out=y[bass.ts(qi, 128), :], in_=out_t)

            pp_f.release()
            ffn_t.release()
            f1T_pool.release()
            wf_pool.release()
            scratch.release()
            pt_pool.release()
            x1T_pool.release()

    nc.finalize()
    return nc


_program_cache = {}


def _get_program():
    if "nc" not in _program_cache:
        _program_cache["nc"] = build_program()
    return _program_cache["nc"]


def kernel(**inputs):
    from concourse.bass_utils import run_bass_kernel_spmd

    x = np.asarray(inputs["x"])  # [4, 2048, 768] f32
    bf = ml_dtypes.bfloat16

    shared = {
        "w_mlp": inputs["w_mlp"].astype(bf),
        "wq": inputs["wq"].astype(bf),
        "wk": inputs["wk"].astype(bf),
        # wv gets a zero row appended; wm gets bm+bv so the on-chip hT
        # ones-row adds the attention-path bias for free
        "wv": np.vstack([inputs["wv"], np.zeros((1, D), np.float32)]).astype(bf),
        "wm": np.vstack([inputs["wm"], (inputs["bm"] + inputs["bv"])[None, :]]).astype(bf),
        "wf1": inputs["wf1"].astype(bf),
        "wf2": inputs["wf2"].astype(bf),
        "b_mlp": inputs["b_mlp"].astype(np.float32),
        "bq": inputs["bq"].astype(np.float32),
        "bk": inputs["bk"].astype(np.float32),
        "bf1": inputs["bf1"].astype(np.float32),
        "bf2": inputs["bf2"].astype(bf),
        "g1": inputs["g1"].astype(bf),
        "be1": inputs["be1"].astype(bf),
        "g2": inputs["g2"].astype(bf),
        "be2": inputs["be2"].astype(bf),
    }

    in_maps = []
    for c in range(NCORES):
        b, half = c // 2, c % 2
        xb = np.roll(x[b], -Q * half, axis=0)  # own half first
        xT = np.ascontiguousarray(xb.T).astype(bf)  # [768, 2048]
        m = dict(shared)
        m["xT"] = xT
        in_maps.append(m)

    nc = _get_program()
    res = run_bass_kernel_spmd(nc, in_maps, core_ids=list(range(NCORES)))

    out = np.empty((B, S, D), np.float32)
    for c in range(NCORES):
        b, half = c // 2, c % 2
        out[b, half * Q : (half + 1) * Q, :] = res.results[c]["y"]
    return out


# revision 49
# speedup vs baseline: 1.0119x; 1.0119x over previous
"""Trainium2 Bass kernel for nn_Joint (dense transformer block), 8 NeuronCores.

Sharding: 8 cores = 4 batches x 2 sequence halves. Each core computes the
full MLP->h and K/V projections for its batch (duplicated inside the pair,
no collectives), but only its own 1024-token half of queries / attention
rows / FFN / output. Token "roll" trick: each core's x is rotated so its own
half is always tokens [0:1024]; attention over all 2048 keys is
permutation-invariant, so the same SPMD program works for both halves.

v2: transposed-scores attention. Scores are computed directly as
S^T = kT.T @ qT (keys on partitions), so P^T = exp(S^T * scale) lands in
SBUF already in the layout attn_out needs as lhsT -- no PE transposes or
PSUM->SBUF copies in the softmax path. Row sums come from an extra N=1
matmul per key-block reusing the loaded stationary operand; softmax skips
max-subtraction (scores*scale bounded by ~0.35 for this model, exp cannot
overflow). LayerNorm math uses fused scalar_tensor_tensor /
tensor_tensor_reduce ops in bf16; the scalar engine runs Exp-only during
attention (Sqrt batched once per 512-query block) to avoid act-table
thrashing. Weight DMAs are prefetched a phase early on the sync queue;
xT streams on gpsimd. SBUF pools use explicit left/right stack lifetimes.

Layouts on chip (per core):
  xT   [768, 2048]  bf16  feature-major (host pre-transposed)
  hT   [568(+1), 2048] bf16 feature-major; row 56 of chunk 4 = ones (bias row)
  kT   [1024, 2048] bf16  feature-major
  qT   [1024, 1024] bf16  feature-major (own half)
  V    [2048, 1024] bf16  token-major
  xmod [1024, 1024] bf16  token-major, bm+bv folded in via the ones row
  PT   [2048, 512]  bf16  exp(S^T*scale) per 512-query block, keys on partitions
  x1   [1024, 1024] bf16  token-major; x1T via PE transpose for FFN
All matmuls bf16 inputs with fp32 PSUM accumulation; softmax/LN math fp32.
"""

import sys

if "/opt/trn_rl_repo" not in sys.path:
    sys.path.insert(0, "/opt/trn_rl_repo")

import numpy as np
import ml_dtypes

import concourse.bass as bass
import concourse.mybir as mybir
import concourse.tile as tile
from concourse import bacc
from concourse.masks import make_identity

BF16 = mybir.dt.bfloat16
F32 = mybir.dt.float32
AF = mybir.ActivationFunctionType
ALU = mybir.AluOpType
AX = mybir.AxisListType

B, S, IN_C, HID, D = 4, 2048, 768, 568, 1024
Q = S // 2  # own-half query tokens per core
KB = S // 128  # 16 key blocks
EPS = 1e-5
SCALE = 1.0 / np.sqrt(np.float32(D))  # 1/32
NCORES = 8

# K-chunking of the HID=568 contraction: 4x128 + 56 (+1 ones row for wm/wv)
HID_CH = [128, 128, 128, 128, 56]
HID_CH_AUG = [128, 128, 128, 128, 57]


def build_program():
    nc = bacc.Bacc("TRN2")

    # ---- DRAM I/O ----
    xT = nc.dram_tensor("xT", [IN_C, S], BF16, kind="ExternalInput")
    w_mlp = nc.dram_tensor("w_mlp", [IN_C, HID], BF16, kind="ExternalInput")
    wq = nc.dram_tensor("wq", [HID, D], BF16, kind="ExternalInput")
    wk = nc.dram_tensor("wk", [HID, D], BF16, kind="ExternalInput")
    # wv/wm are host-augmented to HID+1 rows: wv gets a zero row, wm gets
    # bm+bv -- the hT ones-row turns that into a free bias add
    wv = nc.dram_tensor("wv", [HID + 1, D], BF16, kind="ExternalInput")
    wm = nc.dram_tensor("wm", [HID + 1, D], BF16, kind="ExternalInput")
    wf1 = nc.dram_tensor("wf1", [D, D], BF16, kind="ExternalInput")
    wf2 = nc.dram_tensor("wf2", [D, D], BF16, kind="ExternalInput")
    b_mlp = nc.dram_tensor("b_mlp", [HID], F32, kind="ExternalInput")
    bq = nc.dram_tensor("bq", [D], F32, kind="ExternalInput")
    bk = nc.dram_tensor("bk", [D], F32, kind="ExternalInput")
    bf1 = nc.dram_tensor("bf1", [D], F32, kind="ExternalInput")
    bf2 = nc.dram_tensor("bf2", [D], BF16, kind="ExternalInput")
    g1 = nc.dram_tensor("g1", [D], BF16, kind="ExternalInput")
    be1 = nc.dram_tensor("be1", [D], BF16, kind="ExternalInput")
    g2 = nc.dram_tensor("g2", [D], BF16, kind="ExternalInput")
    be2 = nc.dram_tensor("be2", [D], BF16, kind="ExternalInput")
    y = nc.dram_tensor("y", [Q, D], F32, kind="ExternalOutput")

    def bcast_ap(handle, n):
        a = handle[:]
        return bass.AP(tensor=a.tensor, offset=a.offset, ap=[[0, 128]] + list(a.ap))

    with tile.TileContext(nc) as tc:
        with (
            tc.tile_pool(name="singles", bufs=1) as singles,
            tc.tile_pool(name="x1_pool", bufs=1) as x1_pool,
        ):
            # ---------- constants / biases (small, left stack bottom) ----------
            ident = singles.tile([128, 128], BF16)
            make_identity(nc, ident)
            eps_t = singles.tile([128, 1], F32)
            nc.vector.memset(eps_t, EPS)
            ones_sb = singles.tile([128, 2], BF16)
            nc.vector.memset(ones_sb, 1.0)

            bmlp_sb = singles.tile([128, 5], F32)
            for m in range(5):
                m0 = m * 128
                msz = HID_CH[m]
                nc.sync.dma_start(
                    out=bmlp_sb[:msz, m : m + 1],
                    in_=b_mlp[m0 : m0 + msz].rearrange("(a b) -> a b", b=1),
                )
            bq_sb = singles.tile([128, 8], F32)
            nc.sync.dma_start(out=bq_sb, in_=bq.rearrange("(c p) -> p c", p=128))
            bk_sb = singles.tile([128, 8], F32)
            nc.sync.dma_start(out=bk_sb, in_=bk.rearrange("(c p) -> p c", p=128))
            bf1_sb = singles.tile([128, 8], F32)
            nc.sync.dma_start(out=bf1_sb, in_=bf1.rearrange("(c p) -> p c", p=128))

            # small LN scratch
            mu4 = singles.tile([128, 4], F32)
            sq4 = singles.tile([128, 4], F32)
            ve4 = singles.tile([128, 4], F32)
            msqe4 = singles.tile([128, 4], F32)
            std4 = singles.tile([128, 4], F32)
            rstd4 = singles.tile([128, 4], F32)
            rercp4 = singles.tile([128, 4], F32)
            sa0 = singles.tile([128, 1], F32)
            sa1 = singles.tile([128, 1], F32)
            musum = singles.tile([128, 1], F32)
            bf2sum = singles.tile([128, 1], F32)
            sqa = singles.tile([128, 1], F32)
            mu2 = singles.tile([128, 1], F32)
            msqe2 = singles.tile([128, 1], F32)
            ve2 = singles.tile([128, 1], F32)
            std2 = singles.tile([128, 1], F32)
            rstd2 = singles.tile([128, 1], F32)

            x1_sb = [x1_pool.tile([128, D], BF16, tag=f"x1_{i}", name=f"x1_{i}") for i in range(8)]

            # ---------- pools: left stack (phase-scoped), right stack (late) ----------
            x1T_pool = tc.alloc_tile_pool(name="x1T", bufs=1, side="right")
            x1T = x1T_pool.tile([128, 8, Q], BF16, name="x1T")

            kqvm = tc.alloc_tile_pool(name="kqvm", bufs=1, side="left")
            kT_sb = [kqvm.tile([128, S], BF16, tag=f"kT_{i}", name=f"kT_{i}") for i in range(8)]
            qT_sb = [kqvm.tile([128, Q], BF16, tag=f"qT_{i}", name=f"qT_{i}") for i in range(8)]
            v_sb = [kqvm.tile([128, D], BF16, tag=f"v_{i}", name=f"v_{i}") for i in range(16)]
            xm_sb = [kqvm.tile([128, D], BF16, tag=f"xm_{i}", name=f"xm_{i}") for i in range(8)]

            hT_pool = tc.alloc_tile_pool(name="hT", bufs=1, side="left")
            hT_sb = [hT_pool.tile([128, S], BF16, tag=f"hT_{i}", name=f"hTs_{i}") for i in range(5)]

            stream = tc.alloc_tile_pool(name="stream", bufs=1, side="left")
            xw_pool = tc.alloc_tile_pool(name="xw", bufs=1, side="left")
            wmlp_sb = xw_pool.tile([128, 6, HID], BF16, name="wmlp_sb")

            pp_sc = tc.alloc_tile_pool(name="psum_sc", bufs=1, space="PSUM")
            pp_mm = tc.alloc_tile_pool(name="psum_mm", bufs=1, space="PSUM")

            # ---------- phase 0: hT = relu(w_mlp.T @ xT + b_mlp) ----------
            nc.sync.dma_start(
                out=wmlp_sb, in_=w_mlp.rearrange("(c p) h -> p c h", p=128)
            )
            # ones row for the bias-via-matmul trick (xm / V): memset a
            # 32-aligned partition band; the mlp activations later overwrite
            # rows 32..55, leaving row 56 = 1.0
            nc.gpsimd.memset(hT_sb[4][32:64, :], 1.0)

            # streamed projection weights; two tag sets alternate so the next
            # projection's DMA overlaps the current one's matmuls
            def load_w(wdram, st, chunks=HID_CH):
                tiles = []
                for i in range(5):
                    i0, isz = i * 128, chunks[i]
                    t = stream.tile([128, D], BF16, tag=f"wp{st}_{i}", name=f"wp{st}_{i}")
                    nc.sync.dma_start(out=t[:isz], in_=wdram[i0 : i0 + isz, :])
                    tiles.append(t)
                return tiles

            wk_sb = load_w(wk, 0)
            wq_sb = load_w(wq, 1)

            # xT streamed in 512-column waves, one 3D DMA per wave
            for n in range(4):
                ns = bass.ts(n, 512)
                xs = stream.tile([128, 6, 512], BF16, tag="xs", name="xs", bufs=2)
                for i in range(6):
                    eng = nc.gpsimd if i % 2 == 0 else nc.scalar
                    eng.dma_start(
                        out=xs[:, i, :],
                        in_=xT[i * 128 : (i + 1) * 128, n * 512 : (n + 1) * 512],
                    )
                for m in range(5):
                    m0, msz = m * 128, HID_CH[m]
                    ps = pp_mm.tile([128, 512], F32, tag="mm", bufs=4)
                    for kk in range(6):
                        nc.tensor.matmul(
                            ps[:msz],
                            wmlp_sb[:, kk, m0 : m0 + msz],
                            xs[:, kk, :],
                            start=(kk == 0),
                            stop=(kk == 5),
                        )
                    nc.scalar.activation(
                        out=hT_sb[m][:msz, ns],
                        in_=ps[:msz],
                        func=AF.Relu,
                        bias=bmlp_sb[:msz, m : m + 1],
                    )
            xw_pool.release()

            # ---------- phase 1: projections ----------
            for m in range(8):
                ms = bass.ts(m, 128)
                for n in range(4):
                    ns = bass.ts(n, 512)
                    ps = pp_mm.tile([128, 512], F32, tag="mm", bufs=4)
                    for kk in range(5):
                        ksz = HID_CH[kk]
                        nc.tensor.matmul(
                            ps,
                            wk_sb[kk][:ksz, ms],
                            hT_sb[kk][:ksz, ns],
                            start=(kk == 0),
                            stop=(kk == 4),
                        )
                    nc.scalar.activation(
                        out=kT_sb[m][:, ns], in_=ps, func=AF.Identity,
                        bias=bk_sb[:, m : m + 1],
                    )
            wv_sb = load_w(wv, 0, HID_CH_AUG)  # reuses wk's buffers once kT drains
            for m in range(8):
                ms = bass.ts(m, 128)
                for n in range(2):
                    ns = bass.ts(n, 512)
                    ps = pp_mm.tile([128, 512], F32, tag="mm", bufs=4)
                    for kk in range(5):
                        ksz = HID_CH[kk]
                        nc.tensor.matmul(
                            ps,
                            wq_sb[kk][:ksz, ms],
                            hT_sb[kk][:ksz, ns],
                            start=(kk == 0),
                            stop=(kk == 4),
                        )
                    nc.scalar.activation(
                        out=qT_sb[m][:, ns], in_=ps, func=AF.Identity,
                        bias=bq_sb[:, m : m + 1],
                    )

            def scores_block(b, pt):
                """S^T = kT.T @ qT for 512 queries; P^T = exp(S^T*scale) in SBUF."""
                qs = bass.ts(b, 512)
                for kb in range(KB):
                    ps = pp_sc.tile([128, 512], F32, tag="sc", bufs=2)
                    for kk in range(8):
                        nc.tensor.matmul(
                            ps,
                            kT_sb[kk][:, kb * 128 : (kb + 1) * 128],
                            qT_sb[kk][:, qs],
                            start=(kk == 0),
                            stop=(kk == 7),
                        )
                    nc.scalar.activation(
                        out=pt[:, kb, :], in_=ps, func=AF.Exp, scale=float(SCALE),
                    )

            pt_pool = tc.alloc_tile_pool(name="pt", bufs=1, side="right")
            pt = pt_pool.tile([128, KB, 512], BF16, name="pt")
            scores_block(0, pt)

            # V (token-major): wv row 56 of chunk 4 is zero (host-augmented);
            # wm row 56 carries bm+bv, added via the hT ones row
            wmm_sb = load_w(wm, 1, HID_CH_AUG)
            for m in range(16):
                ms = bass.ts(m, 128)
                for n in range(2):
                    ns = bass.ts(n, 512)
                    ps = pp_mm.tile([128, 512], F32, tag="mm", bufs=4)
                    for kk in range(5):
                        ksz = HID_CH_AUG[kk]
                        nc.tensor.matmul(
                            ps,
                            hT_sb[kk][:ksz, ms],
                            wv_sb[kk][:ksz, ns],
                            start=(kk == 0),
                            stop=(kk == 4),
                        )
                    nc.vector.tensor_copy(v_sb[m][:, ns], ps)
            # xmod (token-major, own half) + (bm+bv) via ones row
            for m in range(8):
                ms = bass.ts(m, 128)
                for n in range(2):
                    ns = bass.ts(n, 512)
                    ps = pp_mm.tile([128, 512], F32, tag="mm", bufs=4)
                    for kk in range(5):
                        ksz = HID_CH_AUG[kk]
                        nc.tensor.matmul(
                            ps,
                            hT_sb[kk][:ksz, ms],
                            wmm_sb[kk][:ksz, ns],
                            start=(kk == 0),
                            stop=(kk == 4),
                        )
                    nc.vector.tensor_copy(xm_sb[m][:, ns], ps)

            stream.release()
            hT_pool.release()
            pp_mm.release()

            # ---------- phase 2: attention ----------
            scratch = tc.alloc_tile_pool(name="scratch", bufs=1, side="right")
            tmpA = scratch.tile([128, D], BF16, name="tmpA")
            tmpB = scratch.tile([128, D], BF16, name="tmpB")
            bf2_b = scratch.tile([128, D], BF16, name="bf2_b")
            nc.gpsimd.dma_start(out=bf2_b, in_=bcast_ap(bf2, D))
            g1_b = scratch.tile([128, D], BF16, name="g1_b")
            nc.gpsimd.dma_start(out=g1_b, in_=bcast_ap(g1, D))
            be1_b = scratch.tile([128, D], BF16, name="be1_b")
            nc.gpsimd.dma_start(out=be1_b, in_=bcast_ap(be1, D))
            g2_b = scratch.tile([128, D], BF16, name="g2_b")
            nc.gpsimd.dma_start(out=g2_b, in_=bcast_ap(g2, D))
            be2_b = scratch.tile([128, D], BF16, name="be2_b")
            nc.gpsimd.dma_start(out=be2_b, in_=bcast_ap(be2, D))

            # precompute sum(bf2) for the LN2 mean correction
            nc.vector.tensor_reduce(
                out=bf2sum, in_=bf2_b, op=ALU.add, axis=AX.X
            )

            wf_pool = tc.alloc_tile_pool(name="wf", bufs=1, side="right")
            wf1_sb = wf_pool.tile([128, 8, D], BF16, name="wf1_sb")
            wf2_sb = wf_pool.tile([128, 8, D], BF16, name="wf2_sb")
            nc.sync.dma_start(
                out=wf1_sb, in_=wf1.rearrange("(c p) d -> p c d", p=128)
            )

            pp_at = tc.alloc_tile_pool(name="psum_at", bufs=1, space="PSUM")
            pp_rs = tc.alloc_tile_pool(name="psum_rs", bufs=1, space="PSUM")

            def attn_block(b, pt):
                for qc in range(4):
                    qi = b * 4 + qc
                    ms = qc * 128
                    ps0 = pp_at.tile([128, 512], F32, tag="at", bufs=3)
                    ps1 = pp_at.tile([128, 512], F32, tag="at", bufs=3)
                    # per-qc rowsum tile: a whole PSUM bank, so the DVE read
                    # below never shares a bank with the next qc's PE writes.
                    # N=2 keeps the matmul output 8-byte-cacheline aligned.
                    rs = pp_rs.tile([128, 2], F32, tag="rs", bufs=2)
                    for kb in range(KB):
                        lhsT = pt[:, kb, ms : ms + 128]
                        nc.tensor.matmul(
                            ps0, lhsT, v_sb[kb][:, 0:512],
                            start=(kb == 0), stop=(kb == KB - 1),
                        )
                        nc.tensor.matmul(
                            ps1, lhsT, v_sb[kb][:, 512:1024],
                            start=(kb == 0), stop=(kb == KB - 1),
                        )
                        nc.tensor.matmul(
                            rs, lhsT, ones_sb,
                            start=(kb == 0), stop=(kb == KB - 1),
                        )
                    nc.vector.reciprocal(rercp4[:, qc : qc + 1], rs[:, 0:1])
                    # x1pre = attn/rowsum + xmod; accum gives the LN mean sum
                    nc.vector.scalar_tensor_tensor(
                        out=x1_sb[qi][:, 0:512], in0=ps0,
                        scalar=rercp4[:, qc : qc + 1], in1=xm_sb[qi][:, 0:512],
                        op0=ALU.mult, op1=ALU.add, accum_out=sa0,
                    )
                    nc.vector.scalar_tensor_tensor(
                        out=x1_sb[qi][:, 512:1024], in0=ps1,
                        scalar=rercp4[:, qc : qc + 1], in1=xm_sb[qi][:, 512:1024],
                        op0=ALU.mult, op1=ALU.add, accum_out=sa1,
                    )
                    nc.vector.tensor_add(musum, sa0, sa1)
                    nc.vector.tensor_scalar_mul(mu4[:, qc : qc + 1], musum, 1.0 / D)
                    # sum of squares on the scalar engine (common act table,
                    # shared with the per-block Sqrt -- still 2 swaps/block)
                    nc.scalar.activation(
                        out=tmpB, in_=x1_sb[qi], func=AF.Square,
                        accum_out=sq4[:, qc : qc + 1],
                    )
                    nc.vector.scalar_tensor_tensor(
                        out=msqe4[:, qc : qc + 1], in0=mu4[:, qc : qc + 1],
                        scalar=mu4[:, qc : qc + 1], in1=eps_t,
                        op0=ALU.mult, op1=ALU.subtract,
                    )
                    nc.vector.scalar_tensor_tensor(
                        out=ve4[:, qc : qc + 1], in0=sq4[:, qc : qc + 1],
                        scalar=1.0 / D, in1=msqe4[:, qc : qc + 1],
                        op0=ALU.mult, op1=ALU.subtract,
                    )
                # batched rstd for the 4 chunks (one act-table swap per block)
                nc.scalar.activation(out=std4, in_=ve4, func=AF.Sqrt)
                nc.vector.reciprocal(rstd4, std4)
                for qc in range(4):
                    qi = b * 4 + qc
                    nc.vector.tensor_scalar(
                        out=tmpA, in0=x1_sb[qi],
                        scalar1=mu4[:, qc : qc + 1], scalar2=rstd4[:, qc : qc + 1],
                        op0=ALU.subtract, op1=ALU.mult,
                    )
                    nc.vector.tensor_mul(tmpB, tmpA, g1_b)
                    nc.vector.tensor_add(x1_sb[qi], tmpB, be1_b)

            def x1T_block(b, psum_pool, tp_bufs):
                for qc in range(4):
                    qi = b * 4 + qc
                    qoff = qi * 128
                    for g in range(2):
                        tp = psum_pool.tile([128, 512], BF16, tag="tp", bufs=tp_bufs)
                        for j in range(4):
                            dj = g * 4 + j
                            nc.tensor.transpose(
                                tp[:, j * 128 : (j + 1) * 128],
                                x1_sb[qi][:, dj * 128 : (dj + 1) * 128],
                                ident,
                            )
                        nc.vector.tensor_copy(
                            x1T[:, g * 4 : (g + 1) * 4, qoff : qoff + 128],
                            tp.rearrange("p (g q) -> p g q", q=128),
                        )

            attn_block(0, pt)
            scores_block(1, pt)
            x1T_block(0, pp_at, 1)
            attn_block(1, pt)
            pp_rs.release()
            pp_at.release()
            pp_sc.release()
            kqvm.release()

            # ---------- phase 3: FFN + LN2 + relu ----------
            pp_f = tc.alloc_tile_pool(name="psum_f", bufs=1, space="PSUM")
            f1T_pool = tc.alloc_tile_pool(name="f1T", bufs=2, side="left")
            ffn_t = tc.alloc_tile_pool(name="ffn_t", bufs=2, side="left")
            nc.sync.dma_start(
                out=wf2_sb, in_=wf2.rearrange("(c p) d -> p c d", p=128)
            )

            first = True
            for nch in range(2):
                f1T_sb = f1T_pool.tile([128, 8, 512], BF16, tag="f1T")
                for m in range(8):
                    ms = bass.ts(m, 128)
                    ps = pp_f.tile([128, 512], F32, tag="f", bufs=4)
                    for kk in range(8):
                        nc.tensor.matmul(
                            ps,
                            wf1_sb[:, kk, ms],
                            x1T[:, kk, nch * 512 : (nch + 1) * 512],
                            start=(kk == 0),
                            stop=(kk == 7),
                        )
                    nc.scalar.activation(
                        out=f1T_sb[:, m, :], in_=ps, func=AF.Relu,
                        bias=bf1_sb[:, m : m + 1],
                    )
                if first:
                    # block-1 x1T transposes, overlapped with f1 of nch 0
                    x1T_block(1, pp_f, 2)
                    first = False
                for tq in range(4):
                    qi = nch * 4 + tq
                    x2a = ffn_t.tile([128, D], BF16, tag="x2a")
                    x2pre = ffn_t.tile([128, D], BF16, tag="x2pre")
                    for dc in range(2):
                        ds_ = bass.ts(dc, 512)
                        ps = pp_f.tile([128, 512], F32, tag="f", bufs=4)
                        for kk in range(8):
                            nc.tensor.matmul(
                                ps,
                                f1T_sb[:, kk, tq * 128 : (tq + 1) * 128],
                                wf2_sb[:, kk, ds_],
                                start=(kk == 0),
                                stop=(kk == 7),
                            )
                        nc.vector.scalar_tensor_tensor(
                            out=x2a[:, ds_], in0=ps, scalar=0.0,
                            in1=x1_sb[qi][:, ds_], op0=ALU.bypass, op1=ALU.add,
                            accum_out=(sa0 if dc == 0 else sa1),
                        )
                    nc.vector.tensor_add(x2pre, x2a, bf2_b)
                    # mean sum = sum(x2a) + sum(bf2)
                    nc.vector.tensor_add(musum, sa0, sa1)
                    nc.vector.tensor_add(musum, musum, bf2sum)
                    nc.vector.tensor_scalar_mul(mu2, musum, 1.0 / D)
                    nc.scalar.activation(
                        out=x2a, in_=x2pre, func=AF.Square, accum_out=sqa,
                    )
                    nc.vector.scalar_tensor_tensor(
                        out=msqe2, in0=mu2, scalar=mu2, in1=eps_t,
                        op0=ALU.mult, op1=ALU.subtract,
                    )
                    nc.vector.scalar_tensor_tensor(
                        out=ve2, in0=sqa, scalar=1.0 / D, in1=msqe2,
                        op0=ALU.mult, op1=ALU.subtract,
                    )
                    nc.scalar.activation(out=std2, in_=ve2, func=AF.Sqrt)
                    nc.vector.reciprocal(rstd2, std2)
                    t1 = ffn_t.tile([128, D], BF16, tag="t1")
                    t2 = ffn_t.tile([128, D], BF16, tag="t2")
                    nc.vector.tensor_scalar(
                        out=t1, in0=x2pre, scalar1=mu2, scalar2=rstd2,
                        op0=ALU.subtract, op1=ALU.mult,
                    )
                    nc.vector.tensor_mul(t2, t1, g2_b)
                    nc.vector.tensor_add(t1, t2, be2_b)
                    out_t = ffn_t.tile([128, D], F32, tag="out")
                    nc.scalar.activation(out=out_t, in_=t1, func=AF.Relu)
                    if qi % 2 == 0:
                        nc.sync.dma_start(out=y[bass.ts(qi, 128), :], in_=out_t)
                    else:
                        nc.scalar.dma_start(out=y[bass.ts(qi, 128), :], in_=out_t)

            pp_f.release()
            ffn_t.release()
            f1T_pool.release()
            wf_pool.release()
            scratch.release()
            pt_pool.release()
            x1T_pool.release()

    nc.finalize()
    return nc


_program_cache = {}


def _get_program():
    if "nc" not in _program_cache:
        _program_cache["nc"] = build_program()
    return _program_cache["nc"]


def kernel(**inputs):
    from concourse.bass_utils import run_bass_kernel_spmd

    x = np.asarray(inputs["x"])  # [4, 2048, 768] f32
    bf = ml_dtypes.bfloat16

    shared = {
        "w_mlp": inputs["w_mlp"].astype(bf),
        "wq": inputs["wq"].astype(bf),
        "wk": inputs["wk"].astype(bf),
        # wv gets a zero row appended; wm gets bm+bv so the on-chip hT
        # ones-row adds the attention-path bias for free
        "wv": np.vstack([inputs["wv"], np.zeros((1, D), np.float32)]).astype(bf),
        "wm": np.vstack([inputs["wm"], (inputs["bm"] + inputs["bv"])[None, :]]).astype(bf),
        "wf1": inputs["wf1"].astype(bf),
        "wf2": inputs["wf2"].astype(bf),
        "b_mlp": inputs["b_mlp"].astype(np.float32),
        "bq": inputs["bq"].astype(np.float32),
        "bk": inputs["bk"].astype(np.float32),
        "bf1": inputs["bf1"].astype(np.float32),
        "bf2": inputs["bf2"].astype(bf),
        "g1": inputs["g1"].astype(bf),
        "be1": inputs["be1"].astype(bf),
        "g2": inputs["g2"].astype(bf),
        "be2": inputs["be2"].astype(bf),
    }

    in_maps = []
    for c in range(NCORES):
        b, half = c // 2, c % 2
        xb = np.roll(x[b], -Q * half, axis=0)  # own half first
        xT = np.ascontiguousarray(xb.T).astype(bf)  # [768, 2048]
        m = dict(shared)
        m["xT"] = xT
        in_maps.append(m)

    nc = _get_program()
    res = run_bass_kernel_spmd(nc, in_maps, core_ids=list(range(NCORES)))

    out = np.empty((B, S, D), np.float32)
    for c in range(NCORES):
        b, half = c // 2, c % 2
        out[b, half * Q : (half + 1) * Q, :] = res.results[c]["y"]
    return out
